# revision 2
# baseline (speedup 1.0000x reference)
"""Self-contained Trainium2 (Bass) kernel for nn_AttentionValueIteration.

Sharding: data-parallel over batch B=8 — one batch element per NeuronCore
(8 cores via jax/axon PJRT). Host-side prep is limited to trivial weight
reshapes/softmax (27x136 elements) and x = values + rewards.

Per-core device pipeline (positions p = t*1024 + h*32 + w, THW = 32768):
  1. x (fp16) DMA'd into a zero-padded [34, 34*34] tile (t on partitions).
  2. 3x3x3 convs as PE matmuls: im2col cols [27, THW] gathered by 27 DMAs
     (tap order (dh, dw, dt)); lhsT = host-pretransposed weights;
     q,k (fp16) and v (bf16) evacuated via 4-chunk PSUM tiles and bounced
     to channel-major DRAM staging with 1057-element zero halos.
  3. DMA rearrange DRAM -> attention layout: partitions = (a*16 + s) for
     s = 16 slabs of 2048 positions; free = (d, pos) for q/k, pos for v.
     k/v carry +-1057 halos, so all 27 neighborhood offsets become plain
     free-dim shifted reads (t-boundary zeros come from the DRAM halo).
  4. 27-offset neighborhood attention: DVE computes prod = q * k_shift
     (fp16, 2x mode) and the 8->1 d-tree; ACT exponentiates (no
     max-subtraction: |sim| < ~75 is safe in bf16); GPSIMD patches
     h/w-boundary wraps (OOB neighbor => sim 0 => e = 1, v-contrib 0)
     and accumulates den/num in bf16, lagged one offset behind DVE.
  5. deninv via DVE reciprocal, qv = num * deninv, max over the 8 actions
     via 3 partition-fold rounds (SBUF-SBUF DMA shift + tensor_max).

The compiled NEFF and the jitted 8-core dispatch are cached module-level;
donated output buffers are recycled so warm calls move only x (fp16 in)
and out (fp16 out) through the PJRT tunnel.
"""
import numpy as np
from contextlib import ExitStack

import concourse.bass as bass
import concourse.tile as tile
from concourse import bacc, mybir

F32 = mybir.dt.float32
F16 = mybir.dt.float16
BF16 = mybir.dt.bfloat16

B, A, D = 8, 8, 8
T = H = W = 32
THW = 32768
NSLAB, SLAB = 16, 2048
HALO = 1057                      # 1024 + 32 + 1
KROW = SLAB + 2 * HALO           # 4162 per-d k/v row with halo
EXT = HALO + THW + HALO          # 34882 staging row
XROW = 34 * 34                   # 1156 padded (h, w) plane
OFFSETS = [(dt, dh, dw) for dt in (-1, 0, 1) for dh in (-1, 0, 1) for dw in (-1, 0, 1)]


def ap(t_ap, offset, dims):
    """Raw AP on the tensor behind an existing AP (flat element space)."""
    return bass.AP(t_ap.tensor, offset, [list(d) for d in dims])


def kernel_body(tc, out_d, x_d, wqkT_d, wvT_d, qk_dram, v_dram):
    nc = tc.nc
    Exp = mybir.ActivationFunctionType.Exp
    with ExitStack() as ctx:
        constp = ctx.enter_context(tc.tile_pool(name="constp", bufs=1))

        # ---- weights: [27, 136] = [q 64 | k 64 | v 8] pre-transposed ----
        w32 = constp.tile([27, 136], F32)
        nc.sync.dma_start(w32[:, 0:128], wqkT_d[:, :])
        nc.sync.dma_start(w32[:, 128:136], wvT_d[:, :])
        w16 = constp.tile([27, 136], F16)
        nc.vector.tensor_copy(w16[:], w32[:])

        with ExitStack() as cctx:
            xpadp = cctx.enter_context(tc.tile_pool(name="xpadp", bufs=1))
            colp = cctx.enter_context(tc.tile_pool(name="colp", bufs=1))
            evacp = cctx.enter_context(tc.tile_pool(name="evacp", bufs=2))
            psump = cctx.enter_context(tc.tile_pool(name="psump", bufs=2, space="PSUM"))

            # ---- x (precomputed host-side), padded [34, 1156] f16 ----
            xpad = xpadp.tile([34, XROW], F16)
            nc.vector.memset(xpad[:], 0)
            interior = [[XROW, 32], [34, 32], [1, 32]]
            int_off = XROW + 34 + 1
            src3d = [[1024, 32], [32, 32], [1, 32]]
            nc.sync.dma_start(ap(xpad[:], int_off, interior), ap(x_d, 0, src3d))

            # ---- DRAM staging halos <- zeros ----
            z128 = xpadp.tile([128, HALO], F16)
            nc.vector.memset(z128[:], 0)
            nc.sync.dma_start(qk_dram[:, 0:HALO], z128[:])
            nc.sync.dma_start(qk_dram[:, HALO + THW:EXT], z128[:])
            zv = z128[0:8, :].bitcast(BF16)
            nc.sync.dma_start(v_dram[:, 0:HALO], zv)
            nc.sync.dma_start(v_dram[:, HALO + THW:EXT], zv)

            # ---- cols: all 27 taps x THW, one DMA per tap row ----
            cols = colp.tile([27, THW], F16)
            for dh in range(3):
                for dw in range(3):
                    for dt in range(3):
                        r = (3 * dh + dw) * 3 + dt
                        eng = nc.sync if r % 2 == 0 else nc.scalar
                        eng.dma_start(
                            ap(cols[:], r * THW, [[THW, 1], [1, THW]]),
                            ap(xpad[:], dt * XROW + dh * 34 + dw,
                               [[XROW, 32], [34, 32], [1, 32]]),
                        )

            # ---- conv pass 1: q+k ([27,128] lhsT), evac to DRAM staging ----
            for g in range(4):
                sq = evacp.tile([128, 8192], F16, tag="sq", bufs=2)
                for j in range(4):
                    psqk = psump.tile([128, 2048], F32, tag="psqk")
                    for i in range(4):
                        c = (16 * g + 4 * j) + i
                        nc.tensor.matmul(psqk[:, i * 512:(i + 1) * 512],
                                         w16[:, 0:128],
                                         cols[:, c * 512:(c + 1) * 512],
                                         start=True, stop=True)
                    eng = nc.vector.tensor_copy if j % 2 == 0 else nc.scalar.copy
                    eng(sq[:, j * 2048:(j + 1) * 2048], psqk[:])
                nc.scalar.dma_start(
                    ap(qk_dram, HALO + 8192 * g, [[EXT, 128], [1, 8192]]), sq[:])

            # ---- conv pass 2: v ([27,8] lhsT) ----
            for g in range(4):
                vst = evacp.tile([8, 8192], BF16, tag="vst", bufs=2)
                for j in range(4):
                    psv = psump.tile([8, 2048], F32, tag="psqk")
                    for i in range(4):
                        c = (16 * g + 4 * j) + i
                        nc.tensor.matmul(psv[:, i * 512:(i + 1) * 512],
                                         w16[:, 128:136],
                                         cols[:, c * 512:(c + 1) * 512],
                                         start=True, stop=True)
                    nc.scalar.copy(vst[:, j * 2048:(j + 1) * 2048], psv[:])
                nc.sync.dma_start(
                    ap(v_dram, HALO + 8192 * g, [[EXT, 8], [1, 8192]]), vst[:])

        # ---- rearrange DRAM -> attention layout ----
        attnp = ctx.enter_context(tc.tile_pool(name="attnp", bufs=1))
        workp = ctx.enter_context(tc.tile_pool(name="workp", bufs=1))

        v_attn = attnp.tile([128, KROW], BF16)
        for a_ in range(A):
            nc.sync.dma_start(
                v_attn[a_ * 16:(a_ + 1) * 16, :],
                ap(v_dram, a_ * EXT, [[SLAB, 16], [1, KROW]]),
            )
        q_attn = attnp.tile([128, D * SLAB], F16)
        k_attn = attnp.tile([128, D * KROW], F16)
        for a_ in range(A):
            # dst iterates (s-partition, d, pos); src row 8a+d, col s*SLAB+pos
            nc.scalar.dma_start(
                ap(q_attn[:], a_ * 16 * D * SLAB,
                   [[D * SLAB, 16], [SLAB, D], [1, SLAB]]),
                ap(qk_dram, 8 * a_ * EXT + HALO, [[SLAB, 16], [EXT, D], [1, SLAB]]),
            )
            nc.sync.dma_start(
                ap(k_attn[:], a_ * 16 * D * KROW,
                   [[D * KROW, 16], [KROW, D], [1, KROW]]),
                ap(qk_dram, (64 + 8 * a_) * EXT, [[SLAB, 16], [EXT, D], [1, KROW]]),
            )

        # ---- attention: 27 offsets ----
        prod = workp.tile([128, D * SLAB], F16)
        sim4 = workp.tile([128, 4 * SLAB], F16)
        es = [workp.tile([128, SLAB], BF16, name=f"e{i}", tag=f"e{i}") for i in range(2)]
        env = workp.tile([128, SLAB], BF16)
        den = workp.tile([128, SLAB], BF16)
        num = workp.tile([128, SLAB], BF16)

        DS, DK = D * SLAB, D * KROW
        q_lo = ap(q_attn[:], 0, [[DS, 128], [SLAB, 4], [1, SLAB]])
        q_hi = ap(q_attn[:], 4 * SLAB, [[DS, 128], [SLAB, 4], [1, SLAB]])

        def wrap_fixes(dt_, dh_, dw_):
            fixes = []
            if dw_ != 0:
                fixes.append((31 if dw_ == 1 else 0, [[SLAB, 128], [32, 64], [1, 1]]))
            if dh_ != 0:
                fixes.append((992 if dh_ == 1 else 0, [[SLAB, 128], [1024, 2], [1, 32]]))
            return fixes

        def accumulate(m):
            """Pool-side accumulation for offset m (lagged one offset)."""
            dt_, dh_, dw_ = OFFSETS[m]
            off = dt_ * 1024 + dh_ * 32 + dw_
            e_m = es[m % 2]
            fixes = wrap_fixes(dt_, dh_, dw_)
            for foff, fdims in fixes:
                nc.gpsimd.memset(ap(e_m[:], foff, fdims), 1.0)
            vsh = ap(v_attn[:], HALO + off, [[KROW, 128], [1, SLAB]])
            nc.gpsimd.tensor_mul(env[:], e_m[:], vsh)
            for foff, fdims in fixes:
                nc.gpsimd.memset(ap(env[:], foff, fdims), 0)
            if m == 0:
                nc.gpsimd.tensor_copy(den[:], e_m[:])
                nc.gpsimd.tensor_copy(num[:], env[:])
            else:
                nc.gpsimd.tensor_add(den[:], den[:], e_m[:])
                nc.gpsimd.tensor_add(num[:], num[:], env[:])

        for n, (dt_, dh_, dw_) in enumerate(OFFSETS):
            off = dt_ * 1024 + dh_ * 32 + dw_
            k_lo = ap(k_attn[:], HALO + off, [[DK, 128], [KROW, 4], [1, SLAB]])
            k_hi = ap(k_attn[:], HALO + off + 4 * KROW, [[DK, 128], [KROW, 4], [1, SLAB]])
            p_lo = ap(prod[:], 0, [[DS, 128], [SLAB, 4], [1, SLAB]])
            p_hi = ap(prod[:], 4 * SLAB, [[DS, 128], [SLAB, 4], [1, SLAB]])
            nc.vector.tensor_mul(p_lo, q_lo, k_lo)
            nc.vector.tensor_mul(p_hi, q_hi, k_hi)
            # d-tree on DVE: 8->4 (sim4), 4->2 (prod lo), 2->1 (sim4 lo)
            nc.vector.tensor_add(sim4[:], prod[:, 0:4 * SLAB], prod[:, 4 * SLAB:])
            nc.vector.tensor_add(prod[:, 0:2 * SLAB], sim4[:, 0:2 * SLAB], sim4[:, 2 * SLAB:])
            nc.vector.tensor_add(sim4[:, 0:SLAB], prod[:, 0:SLAB], prod[:, SLAB:2 * SLAB])
            nc.scalar.activation(es[n % 2][:], sim4[:, 0:SLAB], Exp)
            if n > 0:
                accumulate(n - 1)
        accumulate(len(OFFSETS) - 1)

        # ---- finalize: qv = num/den, max over actions, store ----
        deninv = workp.tile([128, SLAB], F32, tag="prod")
        nc.vector.reciprocal(deninv[:], den[:])
        qv = workp.tile([128, SLAB], F32, tag="sim4")
        nc.vector.tensor_mul(qv[:], num[:], deninv[:])

        tmp = workp.tile([64, SLAB], F32, tag="prod")
        m1 = workp.tile([64, SLAB], F32, tag="den")
        m2 = workp.tile([32, SLAB], F32, tag="num")
        outf = workp.tile([16, SLAB], F16, tag="env")
        nc.sync.dma_start(tmp[:], qv[64:128, :])
        nc.vector.tensor_max(m1[:], qv[0:64, :], tmp[:])
        nc.sync.dma_start(tmp[0:32, :], m1[32:64, :])
        nc.vector.tensor_max(m2[:], m1[0:32, :], tmp[0:32, :])
        nc.sync.dma_start(tmp[0:16, :], m2[16:32, :])
        nc.vector.tensor_max(outf[:], m2[0:16, :], tmp[0:16, :])
        nc.sync.dma_start(ap(out_d, 0, [[SLAB, 16], [1, SLAB]]), outf[:])


def build_nc(num_devices=8, enable_asserts=False):
    nc = bacc.Bacc(
        "TRN2", target_bir_lowering=False, debug=False,
        enable_asserts=enable_asserts, num_devices=num_devices,
    )
    x_d = nc.dram_tensor("x", [THW], F16, kind="ExternalInput").ap()
    wqkT_d = nc.dram_tensor("w_qkT", [27, 128], F32, kind="ExternalInput").ap()
    wvT_d = nc.dram_tensor("w_vT", [27, 8], F32, kind="ExternalInput").ap()
    out_d = nc.dram_tensor("out", [THW], F16, kind="ExternalOutput").ap()
    qk_dram = nc.dram_tensor("qk_stage", [128, EXT], F16, kind="Internal").ap()
    v_dram = nc.dram_tensor("v_stage", [8, EXT], BF16, kind="Internal").ap()
    with tile.TileContext(nc) as tc:
        kernel_body(tc, out_d, x_d, wqkT_d, wvT_d, qk_dram, v_dram)
    nc.compile()
    return nc


def host_prep(w_qk, w_v):
    # tap order (dh, dw, dt) to match the cols gather layout
    wqk = np.asarray(w_qk, np.float32).reshape(128, 3, 3, 3)
    wqk = wqk.transpose(0, 2, 3, 1).reshape(128, 27)
    wqkT = np.ascontiguousarray(wqk.T)
    wv = np.asarray(w_v, np.float32).reshape(8, 27)
    wv = np.exp(wv - wv.max(-1, keepdims=True))
    wv = wv / wv.sum(-1, keepdims=True)
    wv = wv.reshape(8, 3, 3, 3).transpose(0, 2, 3, 1).reshape(8, 27)
    wvT = np.ascontiguousarray(wv.T.astype(np.float32))
    return wqkT, wvT


_CACHE = {}


def _get_runner():
    """Build the NEFF once and wrap it in a cached jitted 8-core dispatch."""
    if "runner" in _CACHE:
        return _CACHE["runner"]
    import jax
    from jax.sharding import Mesh, PartitionSpec
    from jax.experimental.shard_map import shard_map
    from concourse.bass2jax import (
        _bass_exec_p, install_neuronx_cc_hook, partition_id_tensor,
    )

    nc = build_nc()
    install_neuronx_cc_hook()

    in_names, out_names, out_avals, zero_outs = [], [], [], []
    partition_name = nc.partition_id_tensor.name if nc.partition_id_tensor else None
    for alloc in nc.m.functions[0].allocations:
        if not isinstance(alloc, mybir.MemoryLocationSet):
            continue
        name = alloc.memorylocations[0].name
        if alloc.kind == "ExternalInput":
            if name != partition_name:
                in_names.append(name)
        elif alloc.kind == "ExternalOutput":
            out_names.append(name)
            shape = tuple(alloc.tensor_shape)
            dtype = mybir.dt.np(alloc.dtype)
            out_avals.append(jax.core.ShapedArray(shape, dtype))
            zero_outs.append(np.zeros(shape, dtype))
    n_params, n_outs = len(in_names), len(out_avals)
    all_names = in_names + out_names + ([partition_name] if partition_name else [])

    def _body(*args):
        operands = list(args)
        if partition_name is not None:
            operands.append(partition_id_tensor())
        return tuple(_bass_exec_p.bind(
            *operands, out_avals=tuple(out_avals), in_names=tuple(all_names),
            out_names=tuple(out_names), lowering_input_output_aliases=(),
            sim_require_finite=True, sim_require_nnan=True, nc=nc,
        ))

    devices = jax.devices()[:B]
    assert len(devices) == B, f"need {B} neuron cores, have {len(jax.devices())}"
    mesh = Mesh(np.asarray(devices), ("core",))
    donate = tuple(range(n_params, n_params + n_outs))
    sharded = jax.jit(
        shard_map(_body, mesh=mesh,
                  in_specs=(PartitionSpec("core"),) * (n_params + n_outs),
                  out_specs=(PartitionSpec("core"),) * n_outs, check_rep=False),
        donate_argnums=donate, keep_unused=True,
    )

    state = {"prev_out": None}

    def run(x16, wqkT, wvT):
        per_name = {
            "x": np.concatenate([x16[b] for b in range(B)]),
            "w_qkT": np.concatenate([wqkT] * B),
            "w_vT": np.concatenate([wvT] * B),
        }
        concat_in = [per_name[nm] for nm in in_names]
        if state["prev_out"] is not None:
            outs_in = state["prev_out"]  # recycle donated device buffers
        else:
            outs_in = [np.zeros((B * z.shape[0], *z.shape[1:]), z.dtype)
                       for z in zero_outs]
        outs = sharded(*concat_in, *outs_in)
        res = np.asarray(outs[out_names.index("out")])
        state["prev_out"] = list(outs)
        return res.reshape(B, THW)

    _CACHE["runner"] = run
    return run


def kernel(values, rewards, w_qk, w_v):
    values = np.asarray(values, np.float32)
    rewards = np.asarray(rewards, np.float32)
    x16 = (values + rewards).astype(np.float16).reshape(B, THW)
    wqkT, wvT = host_prep(w_qk, w_v)
    run = _get_runner()
    out = run(x16, wqkT, wvT)
    return out.astype(np.float32).reshape(B, 1, T, H, W)


if __name__ == "__main__":
    rng = np.random.default_rng(0)
    o = kernel(
        values=rng.standard_normal((B, 1, T, H, W)).astype(np.float32),
        rewards=rng.standard_normal((B, 1, T, H, W)).astype(np.float32),
        w_qk=(rng.standard_normal((128, 1, 3, 3, 3)) * 0.19245).astype(np.float32),
        w_v=(rng.standard_normal((8, 1, 3, 3, 3)) * 0.19245).astype(np.float32),
    )
    print(o.shape, o.dtype)
